# revision 14
# baseline (speedup 1.0000x reference)
"""Trainium2 Bass kernel for a pre-LN transformer block (MHA + MLP).

Strategy (v2):
  - Data-parallel over batch: 32 batches -> 4 per core x 8 cores.
  - Transposed layout [C, T] on device; all matmul contractions on partitions.
  - ALL matmuls run in bf16 (f32r measured ~4 cyc/row on HW vs bf16 1):
    LN stats come from a host-uploaded bf16 copy of x; psum accumulation is
    f32 so stats stay accurate. Residual stream stays f32.
  - LayerNorm: stats via ones(1/C)-matmul (sums replicated across
    partitions), var = E[x^2]-mu^2, alpha = recip_approx_fast(sqrt(var+eps)),
    h = (x-mu)*alpha emitted straight to bf16.
  - Softmax on S^T tiles [s, t]: exp on ScalarE (fused 1/sqrt(hs) scale,
    bf16 out), causal mask via bf16 triangular mult on the diagonal block,
    denominators ride along as 64 ones-columns in the P^T @ [V|1] matmul;
    normalize with DVE reciprocal_approx_fast + one mult per head.
  - Engine placement: squares/casts on GpSimd (no PSUM access there),
    psum-consuming elementwise on DVE/ScalarE, relu + q/k bias-emission on
    ScalarE (per-partition bias APs), biases bo/b2 as rank-1 matmul rows.
  - PSUM: all accumulators are [128, 768] f32 (2 banks); two tags x bufs=2
    fill the 8 banks.
"""

import numpy as np
import ml_dtypes

import concourse.bass as bass
import concourse.mybir as mybir
import concourse.tile as tile
from concourse.bass_utils import run_bass_kernel_spmd

# ---- problem constants (hardcoded per harness contract) ----
B = 32
T = 768
C = 256
H = 4
HS = 64  # head size
F = 4 * C  # 1024
N_CORES = 8
B_PER_CORE = B // N_CORES  # 4
LN_EPS = 1e-5
F32 = mybir.dt.float32
BF16 = mybir.dt.bfloat16

AF = mybir.ActivationFunctionType
ALU = mybir.AluOpType


def _act_raw(nc, out, in_, func, bias=0.0, scale=1.0):
    """scalar.activation without the Reciprocal/Rsqrt accuracy ban.
    out = func(in_*scale + bias). bias may be an AP only for funcs where
    walrus wants an AP (not Copy/Reciprocal)."""
    eng = nc.scalar
    inputs = [eng.lower_ap(in_)]
    for arg in (bias, scale, 0.0):
        if hasattr(arg, "space"):  # AP
            inputs.append(eng.lower_ap(arg))
        else:
            inputs.append(mybir.ImmediateValue(dtype=mybir.dt.float32,
                                               value=float(arg)))
    return eng.add_instruction(
        mybir.InstActivation(
            name=nc.get_next_instruction_name(),
            func=func,
            ins=inputs,
            outs=[eng.lower_ap(out)],
        )
    )


def chunks512(lo, hi):
    """Chunk [lo, hi) into pieces of at most 512 (single-PSUM-bank cap for
    f32 matmul output), keeping every piece >= 256 wide when possible."""
    out = []
    while hi - lo > 512:
        ln = 512 if hi - lo >= 768 else hi - lo - 256
        out.append((lo, ln))
        lo += ln
    out.append((lo, hi - lo))
    return out


# This walrus build rejects >1 sem wait per instruction (setupSyncWait
# "Too many sync wait commands"). Post-pass: move excess waits onto
# freshly inserted same-engine NoOps immediately before the offender.
_MAX_WAITS = 1


def _split_waits(nc):
    n_new = 0
    for bass_bb in nc.bb_map.values():
        bb = bass_bb.bb
        insts = list(bb.instructions)
        out = []
        changed = False
        for inst in insts:
            si = getattr(inst, "sync_info", None)
            waits = list(si.on_wait) if si and si.on_wait else []
            if len(waits) > _MAX_WAITS:
                changed = True
                excess, keep = waits[:-_MAX_WAITS], waits[-_MAX_WAITS:]
                for j in range(0, len(excess), _MAX_WAITS):
                    nop = mybir.InstNoOp(name=f"waitnop-{n_new}", ins=[], outs=[])
                    n_new += 1
                    nop.engine = inst.engine
                    nop.sync_info = mybir.SyncInfo(
                        on_wait=excess[j:j + _MAX_WAITS], on_update=[])
                    out.append(nop)
                inst.sync_info = mybir.SyncInfo(
                    on_wait=keep, on_update=list(si.on_update))
            out.append(inst)
        if changed:
            bb.instructions = out
    return n_new


def _build_nc():
    nc = bass.Bass("TRN2", target_bir_lowering=False, debug=False,
                   num_devices=N_CORES)

    # ---- DRAM parameters ----
    P = nc.declare_dram_parameter
    xt_d = P("xt", [B_PER_CORE, C, T], F32, isOutput=False)
    xbf_d = P("xbf", [B_PER_CORE, C, T], BF16, isOutput=False)
    wq_d = P("wq", [2, 128, C], BF16, isOutput=False)
    wk_d = P("wk", [2, 128, C], BF16, isOutput=False)
    wv_d = P("wv", [2, 128, C], BF16, isOutput=False)
    wo_d = P("wo", [2, 128, C], BF16, isOutput=False)
    w1_d = P("w1", [2, 128, F], BF16, isOutput=False)
    w2_d = P("w2", [8, 128, C], BF16, isOutput=False)
    bq_d = P("bq", [128, 2], F32, isOutput=False)
    bk_d = P("bk", [128, 2], F32, isOutput=False)
    bv_d = P("bv", [128, C], F32, isOutput=False)
    bo_d = P("bo", [1, C], BF16, isOutput=False)
    b1_d = P("b1", [128, 8], F32, isOutput=False)
    b2_d = P("b2", [1, C], BF16, isOutput=False)
    mask_d = P("mask", [128, 128], BF16, isOutput=False)
    onc_d = P("ones_c", [128, 128], BF16, isOutput=False)
    ont_d = P("ones_t", [1, T], BF16, isOutput=False)
    onv_d = P("ones_va", [128, H, 64], BF16, isOutput=False)
    yt_d = P("yt", [B_PER_CORE, C, T], F32, isOutput=True)

    with tile.TileContext(nc) as tc:
        with (
            tc.tile_pool(name="consts", bufs=1) as consts,
            tc.tile_pool(name="work", bufs=2) as work,
            tc.tile_pool(name="psum", bufs=2, space="PSUM") as psum,
        ):
            _kernel_body(nc, consts, work, psum, xt_d, xbf_d, wq_d, wk_d,
                         wv_d, wo_d, w1_d, w2_d, bq_d, bk_d, bv_d, bo_d,
                         b1_d, b2_d, mask_d, onc_d, ont_d, onv_d, yt_d)
    _split_waits(nc)
    return nc


def _kernel_body(nc, consts, work, psum, xt_d, xbf_d, wq_d, wk_d, wv_d, wo_d,
                 w1_d, w2_d, bq_d, bk_d, bv_d, bo_d, b1_d, b2_d, mask_d,
                 onc_d, ont_d, onv_d, yt_d):
    # ---- load constants ----
    wq_sb = [consts.tile([128, C], BF16, tag=f"wq{i}", name=f"wq{i}") for i in range(2)]
    wk_sb = [consts.tile([128, C], BF16, tag=f"wk{i}", name=f"wk{i}") for i in range(2)]
    wv_sb = [consts.tile([128, C], BF16, tag=f"wv{i}", name=f"wv{i}") for i in range(2)]
    wo_sb = [consts.tile([128, C], BF16, tag=f"wo{i}", name=f"wo{i}") for i in range(2)]
    w1_sb = [consts.tile([128, F], BF16, tag=f"w1{i}", name=f"w1{i}") for i in range(2)]
    w2_sb = [consts.tile([128, C], BF16, tag=f"w2{i}", name=f"w2{i}") for i in range(8)]
    for kt in range(2):
        nc.sync.dma_start(out=wq_sb[kt], in_=wq_d[kt])
        nc.sync.dma_start(out=wk_sb[kt], in_=wk_d[kt])
        nc.sync.dma_start(out=wv_sb[kt], in_=wv_d[kt])
        nc.sync.dma_start(out=wo_sb[kt], in_=wo_d[kt])
        nc.sync.dma_start(out=w1_sb[kt], in_=w1_d[kt])
    for kt in range(8):
        nc.sync.dma_start(out=w2_sb[kt], in_=w2_d[kt])
    bq_sb = consts.tile([128, 2], F32, tag="bq")
    bk_sb = consts.tile([128, 2], F32, tag="bk")
    bv_sb = consts.tile([128, C], F32, tag="bv")
    bo_sb = consts.tile([1, C], BF16, tag="bo")
    b1_sb = consts.tile([128, 8], F32, tag="b1")
    b2_sb = consts.tile([1, C], BF16, tag="b2")
    mask_sb = consts.tile([128, 128], BF16, tag="mask")
    nc.sync.dma_start(out=bq_sb, in_=bq_d[:, :])
    nc.sync.dma_start(out=bk_sb, in_=bk_d[:, :])
    nc.sync.dma_start(out=bv_sb, in_=bv_d[:, :])
    nc.sync.dma_start(out=bo_sb, in_=bo_d[:, :])
    nc.sync.dma_start(out=b1_sb, in_=b1_d[:, :])
    nc.sync.dma_start(out=b2_sb, in_=b2_d[:, :])
    nc.sync.dma_start(out=mask_sb, in_=mask_d[:, :])

    ones_stat = consts.tile([128, 128], BF16, tag="ones_stat")
    nc.sync.dma_start(out=ones_stat, in_=onc_d[:, :])
    ones_row = consts.tile([1, T], BF16, tag="ones_row")
    nc.sync.dma_start(out=ones_row, in_=ont_d[:, :])
    eps_sb = consts.tile([128, 1], F32, tag="eps")
    nc.vector.memset(eps_sb, LN_EPS)

    # [V|1] tiles (double-buffered across batches; ones half pre-written in
    # BOTH buffers so the per-batch refresh never touches it).
    def vaug_tiles(b):
        return [work.tile([128, H, 128], BF16, tag=f"vaug{tt}", bufs=2,
                          name=f"vaug{tt}_{b}") for tt in range(6)]
    for i in range(2):
        for va in vaug_tiles(f"init{i}"):
            nc.sync.dma_start(out=va[:, :, 64:128], in_=onv_d[:, :, :])

    # PSUM tags: pa = 2x[128,T] f32 (4 banks), pss = 2x[128,512] (2 banks),
    # pc = 1x[128,T] (2 banks) reserved for the next batch's LN1+QKV overlap.
    def pa_tile(name):
        return psum.tile([128, T], F32, tag="pa", name=name)

    def pss_tile(width, name):
        return psum.tile([128, width], F32, tag="pss", name=name,
                         padded_shape=[128, 512])

    def pc_tile(shape, name):
        return psum.tile(shape, F32, tag="pc", name=name, bufs=1,
                         padded_shape=[128, T])

    state = {}

    def ln_pieces(b, src_f32, src_bf, tag):
        """Emission pieces for one layer norm through the single-buffer pc
        psum. Leaves 2 bf16 h tiles in state[tag]."""
        sq = [work.tile([128, T], BF16, tag=f"ln_sq{ct}", bufs=2,
                        name=f"{tag}_sq{ct}") for ct in range(2)]
        mu_sb = work.tile([128, T], F32, tag="ln_mu", bufs=2, name=f"{tag}_mu")
        t2 = work.tile([128, T], F32, tag="ln_t2", bufs=2, name=f"{tag}_t2")
        t2m = work.tile([128, T], F32, tag="ln_t2m", bufs=1, name=f"{tag}_t2m")
        lnv = work.tile([128, T], F32, tag="ln_lnv", bufs=1, name=f"{tag}_lnv")
        alpha = work.tile([128, T], F32, tag="ln_al", bufs=2, name=f"{tag}_al")
        h_sb = [work.tile([128, T], BF16, tag=f"ln_h{ct}", bufs=2,
                          name=f"{tag}_h{ct}") for ct in range(2)]
        state[tag] = h_sb

        def p_mu():
            for ct in range(2):
                nc.gpsimd.tensor_tensor(out=sq[ct], in0=src_bf[ct],
                                        in1=src_bf[ct], op=ALU.mult)
            ps = pc_tile([128, T], f"{tag}_psmu")
            for st, ln in chunks512(0, T):
                for kt in range(2):
                    nc.tensor.matmul(ps[:, st:st + ln], ones_stat,
                                     src_bf[kt][:, st:st + ln],
                                     start=(kt == 0), stop=(kt == 1))
            nc.vector.tensor_copy(out=mu_sb, in_=ps)
            nc.scalar.activation(out=t2m, in_=ps, func=AF.Square)

        def p_ex2():
            ps = pc_tile([128, T], f"{tag}_psex2")
            for st, ln in chunks512(0, T):
                for kt in range(2):
                    nc.tensor.matmul(ps[:, st:st + ln], ones_stat,
                                     sq[kt][:, st:st + ln],
                                     start=(kt == 0), stop=(kt == 1))
            nc.vector.tensor_tensor(out=t2, in0=ps, in1=t2m, op=ALU.subtract)
            nc.scalar.activation(out=lnv, in_=t2, func=AF.Ln, bias=eps_sb,
                                 scale=1.0)
            nc.scalar.activation(out=alpha, in_=lnv, func=AF.Exp, scale=-0.5)

        def p_h(ct):
            def run():
                hf = work.tile([128, T], F32, tag=f"ln_hf{ct}", bufs=1,
                               name=f"{tag}_hf{ct}")
                nc.vector.tensor_tensor(out=hf, in0=src_f32[ct], in1=mu_sb,
                                        op=ALU.subtract)
                nc.vector.tensor_tensor(out=h_sb[ct], in0=hf, in1=alpha,
                                        op=ALU.mult)
            return run

        return [p_mu, p_ex2, p_h(0), p_h(1)]

    def qkv_pieces(b):
        """q/k/v projections for batch b out of state[f'ln1_{b}'] via pc."""
        qt = [work.tile([128, T], BF16, tag=f"qt{mt}", bufs=2,
                        name=f"qt{mt}_{b}") for mt in range(2)]
        kt_s = [work.tile([128, T], BF16, tag=f"kt{mt}", bufs=2,
                          name=f"kt{mt}_{b}") for mt in range(2)]
        vaug = vaug_tiles(b)
        state[f"qkv_{b}"] = (qt, kt_s, vaug)

        pieces = []

        def p_qk(w_sb, b_col, dst, mt):
            def run():
                ht = state[f"ln1_{b}"]
                ps = pc_tile([128, T], f"ps_qk{mt}_{b}")
                for st, ln in chunks512(0, T):
                    for kt in range(2):
                        nc.tensor.matmul(
                            ps[:, st:st + ln],
                            w_sb[kt][:, mt * 128:(mt + 1) * 128],
                            ht[kt][:, st:st + ln],
                            start=(kt == 0), stop=(kt == 1))
                nc.scalar.activation(out=dst[mt], in_=ps, func=AF.Identity,
                                     bias=b_col[:, mt:mt + 1], scale=1.0)
            return run

        for mt in range(2):
            pieces.append(p_qk(wq_sb, bq_sb, qt, mt))
        for mt in range(2):
            pieces.append(p_qk(wk_sb, bk_sb, kt_s, mt))

        def p_v(tts):
            def run():
                ht = state[f"ln1_{b}"]
                for tt in tts:
                    ps = pc_tile([128, C], f"ps_v{tt}_{b}")
                    for kt in range(2):
                        nc.tensor.matmul(
                            ps, ht[kt][:, tt * 128:(tt + 1) * 128], wv_sb[kt],
                            start=(kt == 0), stop=(kt == 1))
                    nc.vector.tensor_tensor(
                        out=vaug[tt][:, :, 0:64],
                        in0=ps.rearrange("p (h d) -> p h d", h=H),
                        in1=bv_sb.rearrange("p (h d) -> p h d", h=H),
                        op=ALU.add)
            return run

        for tts in ((0, 1), (2, 3), (4, 5)):
            pieces.append(p_v(tts))
        return pieces

    def load_pieces(b):
        xt = [work.tile([128, T], F32, tag=f"xt{ct}", bufs=2,
                        name=f"xt{ct}_{b}") for ct in range(2)]
        xbf = [work.tile([128, T], BF16, tag=f"xbf{ct}", bufs=2,
                         name=f"xbf{ct}_{b}") for ct in range(2)]
        state[f"x_{b}"] = (xt, xbf)

        def p_load():
            for ct in range(2):
                nc.sync.dma_start(out=xt[ct],
                                  in_=xt_d[b, ct * 128:(ct + 1) * 128, :])
                nc.sync.dma_start(out=xbf[ct],
                                  in_=xbf_d[b, ct * 128:(ct + 1) * 128, :])
        return [p_load]

    def front_pieces(b):
        """Everything for batch b that runs through the pc psum: x load,
        LN1, QKV."""
        if b >= B_PER_CORE:
            return []
        ps = load_pieces(b)
        xt, xbf = state[f"x_{b}"]
        ps += ln_pieces(b, xt, xbf, f"ln1_{b}")
        ps += qkv_pieces(b)
        return ps

    def emit_attention(b, overlap):
        """Attention for batch b; calls one overlap piece per si slot."""
        qt, kt_s, vaug = state[f"qkv_{b}"]
        ot = [work.tile([128, T], BF16, tag=f"ot{mt}", bufs=2,
                        name=f"ot{mt}_{b}") for mt in range(2)]
        state[f"ot_{b}"] = ot
        overlap = list(overlap)
        oi = 0
        for p in range(2):
            pos = [pa_tile(f"ps_po{p}_{hh}_{b}") for hh in range(2)]
            pts = [work.tile([128, T], BF16, tag="pt", bufs=4,
                             name=f"pt{p}_{hh}_{b}") for hh in range(2)]
            for si in range(6):
                lo = si * 128
                for hh in range(2):
                    h, off = 2 * p + hh, hh * 64
                    q_ap = qt[p][off:off + 64, :]
                    k_ap = kt_s[p][off:off + 64, :]
                    pt = pts[hh]
                    for ci, (st, ln) in enumerate(chunks512(lo, T)):
                        ps_s = pss_tile(ln, f"ps_s{h}_{si}_{ci}_{b}")
                        nc.tensor.matmul(ps_s[:, 0:ln],
                                         k_ap[:, lo:lo + 128],
                                         q_ap[:, st:st + ln],
                                         start=True, stop=True)
                        nc.scalar.activation(out=pt[:, st:st + ln],
                                             in_=ps_s[:, 0:ln],
                                             func=AF.Exp, scale=HS ** -0.5)
                        if ci == 0:
                            nc.vector.tensor_tensor(
                                out=pt[:, lo:lo + 128],
                                in0=pt[:, lo:lo + 128],
                                in1=mask_sb, op=ALU.mult)
                        nc.tensor.matmul(pos[hh][:, st:st + ln],
                                         vaug[si][:, h, :],
                                         pt[:, st:st + ln],
                                         start=(si == 0), stop=(si == 5))
                if oi < len(overlap):
                    overlap[oi]()
                    oi += 1
            # normalize pair: ot = o * exp(-ln(l))
            lnl = work.tile([128, T], F32, tag="lnl", bufs=2,
                            name=f"lnl{p}_{b}")
            rbp = work.tile([128, T], F32, tag="rbp", bufs=2,
                            name=f"rbp{p}_{b}")
            for hh in range(2):
                nc.scalar.activation(out=lnl[hh * 64:hh * 64 + 64, :],
                                     in_=pos[hh][64:128, :], func=AF.Ln)
            nc.scalar.activation(out=rbp, in_=lnl, func=AF.Exp, scale=-1.0)
            for hh in range(2):
                off = hh * 64
                nc.vector.tensor_tensor(out=ot[p][off:off + 64, :],
                                        in0=pos[hh][0:64, :],
                                        in1=rbp[off:off + 64, :],
                                        op=ALU.mult)
        while oi < len(overlap):
            overlap[oi]()
            oi += 1

    def emit_back(b):
        """Wo + residual, LN2, MLP, output for batch b (pa/pss psum)."""
        xt, _ = state[f"x_{b}"]
        ot = state[f"ot_{b}"]
        x1 = [work.tile([128, T], F32, tag=f"x1_{ct}", bufs=2,
                        name=f"x1_{ct}_{b}") for ct in range(2)]
        x1bf = [work.tile([128, T], BF16, tag=f"x1bf{ct}", bufs=2,
                          name=f"x1bf{ct}_{b}") for ct in range(2)]
        for mt in range(2):
            ps = pa_tile(f"ps_r{mt}_{b}")
            for st, ln in chunks512(0, T):
                for kt in range(2):
                    nc.tensor.matmul(
                        ps[:, st:st + ln],
                        wo_sb[kt][:, mt * 128:(mt + 1) * 128],
                        ot[kt][:, st:st + ln],
                        start=(kt == 0), stop=False)
                nc.tensor.matmul(
                    ps[:, st:st + ln],
                    bo_sb[0:1, mt * 128:(mt + 1) * 128],
                    ones_row[:, st:st + ln],
                    start=False, stop=True)
            nc.vector.tensor_tensor(out=x1[mt], in0=ps, in1=xt[mt],
                                    op=ALU.add)
            nc.vector.tensor_copy(out=x1bf[mt], in_=x1[mt])

        # LN2 inline via pa (both stat tiles live at once)
        tag = f"ln2_{b}"
        sq = [work.tile([128, T], BF16, tag=f"l2_sq{ct}", bufs=2,
                        name=f"{tag}_sq{ct}") for ct in range(2)]
        for ct in range(2):
            nc.gpsimd.tensor_tensor(out=sq[ct], in0=x1bf[ct], in1=x1bf[ct],
                                    op=ALU.mult)
        ps_mu = pa_tile(f"{tag}_mu")
        ps_ex2 = pa_tile(f"{tag}_ex2")
        for ps, rhs in ((ps_mu, x1bf), (ps_ex2, sq)):
            for st, ln in chunks512(0, T):
                for kt in range(2):
                    nc.tensor.matmul(ps[:, st:st + ln], ones_stat,
                                     rhs[kt][:, st:st + ln],
                                     start=(kt == 0), stop=(kt == 1))
        t2m = work.tile([128, T], F32, tag="l2_t2m", bufs=1, name=f"{tag}_t2m")
        t2 = work.tile([128, T], F32, tag="l2_t2", bufs=2, name=f"{tag}_t2")
        lnv = work.tile([128, T], F32, tag="l2_lnv", bufs=1, name=f"{tag}_lnv")
        alpha = work.tile([128, T], F32, tag="l2_al", bufs=2, name=f"{tag}_al")
        nc.scalar.activation(out=t2m, in_=ps_mu, func=AF.Square)
        nc.vector.tensor_tensor(out=t2, in0=ps_ex2, in1=t2m, op=ALU.subtract)
        nc.scalar.activation(out=lnv, in_=t2, func=AF.Ln, bias=eps_sb,
                             scale=1.0)
        nc.scalar.activation(out=alpha, in_=lnv, func=AF.Exp, scale=-0.5)
        h2 = []
        for ct in range(2):
            hf = work.tile([128, T], F32, tag=f"l2_hf{ct}", bufs=1,
                           name=f"{tag}_hf{ct}")
            hh_sb = work.tile([128, T], BF16, tag=f"l2_h{ct}", bufs=2,
                              name=f"{tag}_h{ct}")
            nc.vector.tensor_tensor(out=hf, in0=x1[ct], in1=ps_mu,
                                    op=ALU.subtract)
            nc.vector.tensor_tensor(out=hh_sb, in0=hf, in1=alpha,
                                    op=ALU.mult)
            h2.append(hh_sb)

        # MLP: ps_u through 1-bank pss chunks, ps_y accumulators in pa
        ps_y = [pa_tile(f"ps_y{mt}_{b}") for mt in range(2)]
        for f in range(8):
            ut = work.tile([128, T], BF16, tag="ut", bufs=3,
                           name=f"ut{f}_{b}")
            for ci, (st, ln) in enumerate(chunks512(0, T)):
                ps_u = pss_tile(ln, f"ps_u{f}_{ci}_{b}")
                for kt in range(2):
                    nc.tensor.matmul(
                        ps_u[:, 0:ln],
                        w1_sb[kt][:, f * 128:(f + 1) * 128],
                        h2[kt][:, st:st + ln],
                        start=(kt == 0), stop=(kt == 1))
                nc.vector.tensor_scalar(out=ut[:, st:st + ln],
                                        in0=ps_u[:, 0:ln],
                                        scalar1=b1_sb[:, f:f + 1],
                                        scalar2=0.0,
                                        op0=ALU.add, op1=ALU.max)
            for mt in range(2):
                for st, ln in chunks512(0, T):
                    nc.tensor.matmul(
                        ps_y[mt][:, st:st + ln],
                        w2_sb[f][:, mt * 128:(mt + 1) * 128],
                        ut[:, st:st + ln],
                        start=(f == 0), stop=False)
        yt = [work.tile([128, T], F32, tag=f"yt{ct}", bufs=2,
                        name=f"yt{ct}_{b}") for ct in range(2)]
        for mt in range(2):
            for st, ln in chunks512(0, T):
                nc.tensor.matmul(
                    ps_y[mt][:, st:st + ln],
                    b2_sb[0:1, mt * 128:(mt + 1) * 128],
                    ones_row[:, st:st + ln],
                    start=False, stop=True)
            nc.vector.tensor_tensor(out=yt[mt], in0=ps_y[mt], in1=x1[mt],
                                    op=ALU.add)
            nc.sync.dma_start(out=yt_d[b, mt * 128:(mt + 1) * 128, :],
                              in_=yt[mt])

    # ---- software pipeline over the 4 batches ----
    for piece in front_pieces(0):
        piece()
    for b in range(B_PER_CORE):
        with nc.named_scope(f"attn_{b}"):
            emit_attention(b, front_pieces(b + 1))
        with nc.named_scope(f"back_{b}"):
            emit_back(b)


_NC_CACHE = None


def _prep_weights(Wq, Wk, Wv, Wo, bo, W1, b1, W2, b2, g1, be1, g2, be2):
    f64 = np.float64
    g1, be1 = g1.astype(f64), be1.astype(f64)
    g2, be2 = g2.astype(f64), be2.astype(f64)

    def fold_qkv(W):  # W: [H, C, HS] -> folded [C, H*HS], bias [H*HS]
        Wf = W.astype(f64) * g1[None, :, None]
        Wcat = np.concatenate([Wf[h] for h in range(H)], axis=1)  # [C, 256]
        bias = np.concatenate([be1 @ Wf[h] for h in range(H)])  # [256]
        return Wcat, bias

    WqF, bq = fold_qkv(Wq)
    WkF, bk = fold_qkv(Wk)
    WvF, bv = fold_qkv(Wv)
    # h2 = z*g2 + be2 ; relu(h2@W1 + b1) = relu(z @ (g2*W1) + (be2@W1 + b1))
    W1F = W1.astype(f64) * g2[:, None]
    b1F = b1.astype(f64) + be2 @ W1.astype(f64)

    def f32(a):
        return np.ascontiguousarray(a, dtype=np.float32)

    def bf16(a):
        return np.ascontiguousarray(np.asarray(a, f64).astype(ml_dtypes.bfloat16))

    return {
        "wq": bf16(WqF.reshape(2, 128, C)),
        "wk": bf16(WkF.reshape(2, 128, C)),
        "wv": bf16(WvF.reshape(2, 128, C)),
        "wo": bf16(np.asarray(Wo, f64).reshape(2, 128, C)),
        "w1": bf16(W1F.reshape(2, 128, F)),
        "w2": bf16(np.asarray(W2, f64).reshape(8, 128, C)),
        "bq": f32(bq.reshape(2, 128).T),
        "bk": f32(bk.reshape(2, 128).T),
        "bv": f32(np.broadcast_to(bv, (128, C))),
        "bo": bf16(np.asarray(bo, f64).reshape(1, C)),
        "b1": f32(b1F.reshape(8, 128).T),
        "b2": bf16(np.asarray(b2, f64).reshape(1, C)),
        "mask": bf16(np.triu(np.ones((128, 128)))),
        "ones_c": bf16(np.full((128, 128), 1.0 / C)),
        "ones_t": bf16(np.ones((1, T))),
        "ones_va": bf16(np.ones((128, H, 64))),
    }


def kernel(x, Wq, Wk, Wv, Wo, bo, W1, b1, W2, b2, g1, be1, g2, be2,
           _trace=False):
    global _NC_CACHE
    if _NC_CACHE is None:
        _NC_CACHE = _build_nc()
    nc = _NC_CACHE

    x = np.asarray(x, dtype=np.float32)
    weights = _prep_weights(
        np.asarray(Wq), np.asarray(Wk), np.asarray(Wv), np.asarray(Wo),
        np.asarray(bo), np.asarray(W1), np.asarray(b1), np.asarray(W2),
        np.asarray(b2), np.asarray(g1), np.asarray(be1), np.asarray(g2),
        np.asarray(be2))
    xt = np.ascontiguousarray(x.transpose(0, 2, 1))  # [B, C, T]
    xbf = xt.astype(ml_dtypes.bfloat16)

    in_maps = []
    for core in range(N_CORES):
        m = dict(weights)
        sl = slice(core * B_PER_CORE, (core + 1) * B_PER_CORE)
        m["xt"] = np.ascontiguousarray(xt[sl])
        m["xbf"] = np.ascontiguousarray(xbf[sl])
        in_maps.append(m)

    res = run_bass_kernel_spmd(nc, in_maps, list(range(N_CORES)),
                               trace=_trace)
    outs = [res.results[i]["yt"] for i in range(N_CORES)]  # [4, C, T] each
    y = np.concatenate(outs, axis=0).transpose(0, 2, 1)  # [B, T, C]
    if _trace:
        kernel.last_exec_time_ns = res.exec_time_ns
        kernel.last_results = res
    return np.ascontiguousarray(y)


# revision 15
# speedup vs baseline: 1.0147x; 1.0147x over previous
"""Trainium2 Bass kernel for a pre-LN transformer block (MHA + MLP).

Strategy (v2):
  - Data-parallel over batch: 32 batches -> 4 per core x 8 cores.
  - Transposed layout [C, T] on device; all matmul contractions on partitions.
  - ALL matmuls run in bf16 (f32r measured ~4 cyc/row on HW vs bf16 1):
    LN stats come from a host-uploaded bf16 copy of x; psum accumulation is
    f32 so stats stay accurate. Residual stream stays f32.
  - LayerNorm: stats via ones(1/C)-matmul (sums replicated across
    partitions), var = E[x^2]-mu^2, alpha = recip_approx_fast(sqrt(var+eps)),
    h = (x-mu)*alpha emitted straight to bf16.
  - Softmax on S^T tiles [s, t]: exp on ScalarE (fused 1/sqrt(hs) scale,
    bf16 out), causal mask via bf16 triangular mult on the diagonal block,
    denominators ride along as 64 ones-columns in the P^T @ [V|1] matmul;
    normalize with DVE reciprocal_approx_fast + one mult per head.
  - Engine placement: squares/casts on GpSimd (no PSUM access there),
    psum-consuming elementwise on DVE/ScalarE, relu + q/k bias-emission on
    ScalarE (per-partition bias APs), biases bo/b2 as rank-1 matmul rows.
  - PSUM: all accumulators are [128, 768] f32 (2 banks); two tags x bufs=2
    fill the 8 banks.
"""

import numpy as np
import ml_dtypes

import concourse.bass as bass
import concourse.mybir as mybir
import concourse.tile as tile
from concourse.bass_utils import run_bass_kernel_spmd

# ---- problem constants (hardcoded per harness contract) ----
B = 32
T = 768
C = 256
H = 4
HS = 64  # head size
F = 4 * C  # 1024
N_CORES = 8
B_PER_CORE = B // N_CORES  # 4
LN_EPS = 1e-5
F32 = mybir.dt.float32
BF16 = mybir.dt.bfloat16

AF = mybir.ActivationFunctionType
ALU = mybir.AluOpType


def _act_raw(nc, out, in_, func, bias=0.0, scale=1.0):
    """scalar.activation without the Reciprocal/Rsqrt accuracy ban.
    out = func(in_*scale + bias). bias may be an AP only for funcs where
    walrus wants an AP (not Copy/Reciprocal)."""
    eng = nc.scalar
    inputs = [eng.lower_ap(in_)]
    for arg in (bias, scale, 0.0):
        if hasattr(arg, "space"):  # AP
            inputs.append(eng.lower_ap(arg))
        else:
            inputs.append(mybir.ImmediateValue(dtype=mybir.dt.float32,
                                               value=float(arg)))
    return eng.add_instruction(
        mybir.InstActivation(
            name=nc.get_next_instruction_name(),
            func=func,
            ins=inputs,
            outs=[eng.lower_ap(out)],
        )
    )


def chunks512(lo, hi):
    """Chunk [lo, hi) into pieces of at most 512 (single-PSUM-bank cap for
    f32 matmul output), keeping every piece >= 256 wide when possible."""
    out = []
    while hi - lo > 512:
        ln = 512 if hi - lo >= 768 else hi - lo - 256
        out.append((lo, ln))
        lo += ln
    out.append((lo, hi - lo))
    return out


# This walrus build rejects >1 sem wait per instruction (setupSyncWait
# "Too many sync wait commands"). Post-pass: move excess waits onto
# freshly inserted same-engine NoOps immediately before the offender.
_MAX_WAITS = 1


def _split_waits(nc):
    n_new = 0
    for bass_bb in nc.bb_map.values():
        bb = bass_bb.bb
        insts = list(bb.instructions)
        out = []
        changed = False
        for inst in insts:
            si = getattr(inst, "sync_info", None)
            waits = list(si.on_wait) if si and si.on_wait else []
            if len(waits) > _MAX_WAITS:
                changed = True
                excess, keep = waits[:-_MAX_WAITS], waits[-_MAX_WAITS:]
                for j in range(0, len(excess), _MAX_WAITS):
                    nop = mybir.InstNoOp(name=f"waitnop-{n_new}", ins=[], outs=[])
                    n_new += 1
                    nop.engine = inst.engine
                    nop.sync_info = mybir.SyncInfo(
                        on_wait=excess[j:j + _MAX_WAITS], on_update=[])
                    out.append(nop)
                inst.sync_info = mybir.SyncInfo(
                    on_wait=keep, on_update=list(si.on_update))
            out.append(inst)
        if changed:
            bb.instructions = out
    return n_new


def _build_nc():
    nc = bass.Bass("TRN2", target_bir_lowering=False, debug=False,
                   num_devices=N_CORES)

    # ---- DRAM parameters ----
    P = nc.declare_dram_parameter
    xt_d = P("xt", [B_PER_CORE, C, T], F32, isOutput=False)
    xbf_d = P("xbf", [B_PER_CORE, C, T], BF16, isOutput=False)
    wq_d = P("wq", [2, 128, C], BF16, isOutput=False)
    wk_d = P("wk", [2, 128, C], BF16, isOutput=False)
    wv_d = P("wv", [2, 128, C], BF16, isOutput=False)
    wo_d = P("wo", [2, 128, C], BF16, isOutput=False)
    w1_d = P("w1", [2, 128, F], BF16, isOutput=False)
    w2_d = P("w2", [8, 128, C], BF16, isOutput=False)
    bq_d = P("bq", [128, 2], F32, isOutput=False)
    bk_d = P("bk", [128, 2], F32, isOutput=False)
    bv_d = P("bv", [128, C], F32, isOutput=False)
    bo_d = P("bo", [1, C], BF16, isOutput=False)
    b1_d = P("b1", [128, 8], F32, isOutput=False)
    b2_d = P("b2", [1, C], BF16, isOutput=False)
    mask_d = P("mask", [128, 128], BF16, isOutput=False)
    onc_d = P("ones_c", [128, 128], BF16, isOutput=False)
    ont_d = P("ones_t", [1, T], BF16, isOutput=False)
    onv_d = P("ones_va", [128, H, 64], BF16, isOutput=False)
    yt_d = P("yt", [B_PER_CORE, C, T], F32, isOutput=True)

    with tile.TileContext(nc) as tc:
        with (
            tc.tile_pool(name="consts", bufs=1) as consts,
            tc.tile_pool(name="work", bufs=2) as work,
            tc.tile_pool(name="psum", bufs=2, space="PSUM") as psum,
        ):
            _kernel_body(nc, consts, work, psum, xt_d, xbf_d, wq_d, wk_d,
                         wv_d, wo_d, w1_d, w2_d, bq_d, bk_d, bv_d, bo_d,
                         b1_d, b2_d, mask_d, onc_d, ont_d, onv_d, yt_d)
    _split_waits(nc)
    return nc


def _kernel_body(nc, consts, work, psum, xt_d, xbf_d, wq_d, wk_d, wv_d, wo_d,
                 w1_d, w2_d, bq_d, bk_d, bv_d, bo_d, b1_d, b2_d, mask_d,
                 onc_d, ont_d, onv_d, yt_d):
    # ---- load constants ----
    wq_sb = [consts.tile([128, C], BF16, tag=f"wq{i}", name=f"wq{i}") for i in range(2)]
    wk_sb = [consts.tile([128, C], BF16, tag=f"wk{i}", name=f"wk{i}") for i in range(2)]
    wv_sb = [consts.tile([128, C], BF16, tag=f"wv{i}", name=f"wv{i}") for i in range(2)]
    wo_sb = [consts.tile([128, C], BF16, tag=f"wo{i}", name=f"wo{i}") for i in range(2)]
    w1_sb = [consts.tile([128, F], BF16, tag=f"w1{i}", name=f"w1{i}") for i in range(2)]
    w2_sb = [consts.tile([128, C], BF16, tag=f"w2{i}", name=f"w2{i}") for i in range(8)]
    for kt in range(2):
        nc.sync.dma_start(out=wq_sb[kt], in_=wq_d[kt])
        nc.sync.dma_start(out=wk_sb[kt], in_=wk_d[kt])
        nc.sync.dma_start(out=wv_sb[kt], in_=wv_d[kt])
        nc.sync.dma_start(out=wo_sb[kt], in_=wo_d[kt])
        nc.sync.dma_start(out=w1_sb[kt], in_=w1_d[kt])
    for kt in range(8):
        nc.sync.dma_start(out=w2_sb[kt], in_=w2_d[kt])
    bq_sb = consts.tile([128, 2], F32, tag="bq")
    bk_sb = consts.tile([128, 2], F32, tag="bk")
    bv_sb = consts.tile([128, C], F32, tag="bv")
    bo_sb = consts.tile([1, C], BF16, tag="bo")
    b1_sb = consts.tile([128, 8], F32, tag="b1")
    b2_sb = consts.tile([1, C], BF16, tag="b2")
    mask_sb = consts.tile([128, 128], BF16, tag="mask")
    nc.sync.dma_start(out=bq_sb, in_=bq_d[:, :])
    nc.sync.dma_start(out=bk_sb, in_=bk_d[:, :])
    nc.sync.dma_start(out=bv_sb, in_=bv_d[:, :])
    nc.sync.dma_start(out=bo_sb, in_=bo_d[:, :])
    nc.sync.dma_start(out=b1_sb, in_=b1_d[:, :])
    nc.sync.dma_start(out=b2_sb, in_=b2_d[:, :])
    nc.sync.dma_start(out=mask_sb, in_=mask_d[:, :])

    ones_stat = consts.tile([128, 128], BF16, tag="ones_stat")
    nc.sync.dma_start(out=ones_stat, in_=onc_d[:, :])
    ones_row = consts.tile([1, T], BF16, tag="ones_row")
    nc.sync.dma_start(out=ones_row, in_=ont_d[:, :])
    eps_sb = consts.tile([128, 1], F32, tag="eps")
    nc.vector.memset(eps_sb, LN_EPS)

    # [V|1] tiles (double-buffered across batches; ones half pre-written in
    # BOTH buffers so the per-batch refresh never touches it).
    def vaug_tiles(b):
        return [work.tile([128, H, 128], BF16, tag=f"vaug{tt}", bufs=2,
                          name=f"vaug{tt}_{b}") for tt in range(6)]
    for i in range(2):
        for va in vaug_tiles(f"init{i}"):
            nc.sync.dma_start(out=va[:, :, 64:128], in_=onv_d[:, :, :])

    # PSUM tags: pa = 2x[128,T] f32 (4 banks), pss = 2x[128,512] (2 banks),
    # pc = 1x[128,T] (2 banks) reserved for the next batch's LN1+QKV overlap.
    def pa_tile(name):
        return psum.tile([128, T], F32, tag="pa", name=name)

    def pss_tile(width, name):
        return psum.tile([128, width], F32, tag="pss", name=name,
                         padded_shape=[128, 512])

    def pc_tile(shape, name):
        return psum.tile(shape, F32, tag="pc", name=name, bufs=1,
                         padded_shape=[128, T])

    state = {}

    def ln_pieces(b, src_f32, src_bf, tag):
        """Emission pieces for one layer norm through the single-buffer pc
        psum. Leaves 2 bf16 h tiles in state[tag]."""
        sq = [work.tile([128, T], BF16, tag=f"ln_sq{ct}", bufs=2,
                        name=f"{tag}_sq{ct}") for ct in range(2)]
        mu_sb = work.tile([128, T], F32, tag="ln_mu", bufs=2, name=f"{tag}_mu")
        t2 = work.tile([128, T], F32, tag="ln_t2", bufs=2, name=f"{tag}_t2")
        t2m = work.tile([128, T], F32, tag="ln_t2m", bufs=1, name=f"{tag}_t2m")
        lnv = work.tile([128, T], F32, tag="ln_lnv", bufs=1, name=f"{tag}_lnv")
        alpha = work.tile([128, T], F32, tag="ln_al", bufs=2, name=f"{tag}_al")
        h_sb = [work.tile([128, T], BF16, tag=f"ln_h{ct}", bufs=2,
                          name=f"{tag}_h{ct}") for ct in range(2)]
        state[tag] = h_sb

        def p_mu():
            for ct in range(2):
                nc.gpsimd.tensor_tensor(out=sq[ct], in0=src_bf[ct],
                                        in1=src_bf[ct], op=ALU.mult)
            ps = pc_tile([128, T], f"{tag}_psmu")
            for st, ln in chunks512(0, T):
                for kt in range(2):
                    nc.tensor.matmul(ps[:, st:st + ln], ones_stat,
                                     src_bf[kt][:, st:st + ln],
                                     start=(kt == 0), stop=(kt == 1))
            nc.vector.tensor_copy(out=mu_sb, in_=ps)
            nc.scalar.activation(out=t2m, in_=ps, func=AF.Square)

        def p_ex2():
            ps = pc_tile([128, T], f"{tag}_psex2")
            for st, ln in chunks512(0, T):
                for kt in range(2):
                    nc.tensor.matmul(ps[:, st:st + ln], ones_stat,
                                     sq[kt][:, st:st + ln],
                                     start=(kt == 0), stop=(kt == 1))
            nc.vector.tensor_tensor(out=t2, in0=ps, in1=t2m, op=ALU.subtract)
            nc.scalar.activation(out=lnv, in_=t2, func=AF.Ln, bias=eps_sb,
                                 scale=1.0)
            nc.scalar.activation(out=alpha, in_=lnv, func=AF.Exp, scale=-0.5)

        def p_h(ct):
            def run():
                hf = work.tile([128, T], F32, tag=f"ln_hf{ct}", bufs=1,
                               name=f"{tag}_hf{ct}")
                nc.vector.tensor_tensor(out=hf, in0=src_f32[ct], in1=mu_sb,
                                        op=ALU.subtract)
                nc.vector.tensor_tensor(out=h_sb[ct], in0=hf, in1=alpha,
                                        op=ALU.mult)
            return run

        return [p_mu, p_ex2, p_h(0), p_h(1)]

    def qkv_pieces(b):
        """q/k/v projections for batch b out of state[f'ln1_{b}'] via pc."""
        qt = [work.tile([128, T], BF16, tag=f"qt{mt}", bufs=2,
                        name=f"qt{mt}_{b}") for mt in range(2)]
        kt_s = [work.tile([128, T], BF16, tag=f"kt{mt}", bufs=2,
                          name=f"kt{mt}_{b}") for mt in range(2)]
        vaug = vaug_tiles(b)
        state[f"qkv_{b}"] = (qt, kt_s, vaug)

        pieces = []

        def p_qk(w_sb, b_col, dst, mt):
            def run():
                ht = state[f"ln1_{b}"]
                ps = pc_tile([128, T], f"ps_qk{mt}_{b}")
                for st, ln in chunks512(0, T):
                    for kt in range(2):
                        nc.tensor.matmul(
                            ps[:, st:st + ln],
                            w_sb[kt][:, mt * 128:(mt + 1) * 128],
                            ht[kt][:, st:st + ln],
                            start=(kt == 0), stop=(kt == 1))
                nc.scalar.activation(out=dst[mt], in_=ps, func=AF.Identity,
                                     bias=b_col[:, mt:mt + 1], scale=1.0)
            return run

        for mt in range(2):
            pieces.append(p_qk(wq_sb, bq_sb, qt, mt))
        for mt in range(2):
            pieces.append(p_qk(wk_sb, bk_sb, kt_s, mt))

        def p_v(tts):
            def run():
                ht = state[f"ln1_{b}"]
                for tt in tts:
                    ps = pc_tile([128, C], f"ps_v{tt}_{b}")
                    for kt in range(2):
                        nc.tensor.matmul(
                            ps, ht[kt][:, tt * 128:(tt + 1) * 128], wv_sb[kt],
                            start=(kt == 0), stop=(kt == 1))
                    nc.vector.tensor_tensor(
                        out=vaug[tt][:, :, 0:64],
                        in0=ps.rearrange("p (h d) -> p h d", h=H),
                        in1=bv_sb.rearrange("p (h d) -> p h d", h=H),
                        op=ALU.add)
            return run

        for tts in ((0, 1), (2, 3), (4, 5)):
            pieces.append(p_v(tts))
        return pieces

    def load_pieces(b):
        xt = [work.tile([128, T], F32, tag=f"xt{ct}", bufs=2,
                        name=f"xt{ct}_{b}") for ct in range(2)]
        xbf = [work.tile([128, T], BF16, tag=f"xbf{ct}", bufs=2,
                         name=f"xbf{ct}_{b}") for ct in range(2)]
        state[f"x_{b}"] = (xt, xbf)

        def p_load():
            for ct in range(2):
                nc.sync.dma_start(out=xt[ct],
                                  in_=xt_d[b, ct * 128:(ct + 1) * 128, :])
                nc.sync.dma_start(out=xbf[ct],
                                  in_=xbf_d[b, ct * 128:(ct + 1) * 128, :])
        return [p_load]

    def front_pieces(b):
        """Everything for batch b that runs through the pc psum: x load,
        LN1, QKV."""
        if b >= B_PER_CORE:
            return []
        ps = load_pieces(b)
        xt, xbf = state[f"x_{b}"]
        ps += ln_pieces(b, xt, xbf, f"ln1_{b}")
        ps += qkv_pieces(b)
        return ps

    def emit_attention(b):
        """Attention for batch b."""
        qt, kt_s, vaug = state[f"qkv_{b}"]
        ot = [work.tile([128, T], BF16, tag=f"ot{mt}", bufs=2,
                        name=f"ot{mt}_{b}") for mt in range(2)]
        state[f"ot_{b}"] = ot
        for p in range(2):
            pos = [pa_tile(f"ps_po{p}_{hh}_{b}") for hh in range(2)]
            pts = [work.tile([128, T], BF16, tag="pt", bufs=4,
                             name=f"pt{p}_{hh}_{b}") for hh in range(2)]
            for si in range(6):
                lo = si * 128
                for hh in range(2):
                    h, off = 2 * p + hh, hh * 64
                    q_ap = qt[p][off:off + 64, :]
                    k_ap = kt_s[p][off:off + 64, :]
                    pt = pts[hh]
                    for ci, (st, ln) in enumerate(chunks512(lo, T)):
                        ps_s = pss_tile(ln, f"ps_s{h}_{si}_{ci}_{b}")
                        nc.tensor.matmul(ps_s[:, 0:ln],
                                         k_ap[:, lo:lo + 128],
                                         q_ap[:, st:st + ln],
                                         start=True, stop=True)
                        nc.scalar.activation(out=pt[:, st:st + ln],
                                             in_=ps_s[:, 0:ln],
                                             func=AF.Exp, scale=HS ** -0.5)
                        if ci == 0:
                            nc.vector.tensor_tensor(
                                out=pt[:, lo:lo + 128],
                                in0=pt[:, lo:lo + 128],
                                in1=mask_sb, op=ALU.mult)
                        nc.tensor.matmul(pos[hh][:, st:st + ln],
                                         vaug[si][:, h, :],
                                         pt[:, st:st + ln],
                                         start=(si == 0), stop=(si == 5))
            # normalize pair: ot = o * exp(-ln(l))
            lnl = work.tile([128, T], F32, tag="lnl", bufs=2,
                            name=f"lnl{p}_{b}")
            rbp = work.tile([128, T], F32, tag="rbp", bufs=2,
                            name=f"rbp{p}_{b}")
            for hh in range(2):
                nc.scalar.activation(out=lnl[hh * 64:hh * 64 + 64, :],
                                     in_=pos[hh][64:128, :], func=AF.Ln)
            nc.scalar.activation(out=rbp, in_=lnl, func=AF.Exp, scale=-1.0)
            for hh in range(2):
                off = hh * 64
                nc.vector.tensor_tensor(out=ot[p][off:off + 64, :],
                                        in0=pos[hh][0:64, :],
                                        in1=rbp[off:off + 64, :],
                                        op=ALU.mult)

    def emit_back(b, overlap):
        """Wo + residual, LN2, MLP, output for batch b (pa/pss psum).
        Interleaves the next batch's pc-based front pieces into the slots
        between matmul groups (scalar engine is quiet here)."""
        overlap = list(overlap)
        oi = [0]

        def slot():
            if oi[0] < len(overlap):
                overlap[oi[0]]()
                oi[0] += 1
        xt, _ = state[f"x_{b}"]
        ot = state[f"ot_{b}"]
        x1 = [work.tile([128, T], F32, tag=f"x1_{ct}", bufs=2,
                        name=f"x1_{ct}_{b}") for ct in range(2)]
        x1bf = [work.tile([128, T], BF16, tag=f"x1bf{ct}", bufs=2,
                          name=f"x1bf{ct}_{b}") for ct in range(2)]
        for mt in range(2):
            ps = pa_tile(f"ps_r{mt}_{b}")
            for st, ln in chunks512(0, T):
                for kt in range(2):
                    nc.tensor.matmul(
                        ps[:, st:st + ln],
                        wo_sb[kt][:, mt * 128:(mt + 1) * 128],
                        ot[kt][:, st:st + ln],
                        start=(kt == 0), stop=False)
                nc.tensor.matmul(
                    ps[:, st:st + ln],
                    bo_sb[0:1, mt * 128:(mt + 1) * 128],
                    ones_row[:, st:st + ln],
                    start=False, stop=True)
            nc.vector.tensor_tensor(out=x1[mt], in0=ps, in1=xt[mt],
                                    op=ALU.add)
            nc.vector.tensor_copy(out=x1bf[mt], in_=x1[mt])
            slot()

        # LN2 inline via pa (both stat tiles live at once)
        tag = f"ln2_{b}"
        sq = [work.tile([128, T], BF16, tag=f"l2_sq{ct}", bufs=2,
                        name=f"{tag}_sq{ct}") for ct in range(2)]
        for ct in range(2):
            nc.gpsimd.tensor_tensor(out=sq[ct], in0=x1bf[ct], in1=x1bf[ct],
                                    op=ALU.mult)
        ps_mu = pa_tile(f"{tag}_mu")
        ps_ex2 = pa_tile(f"{tag}_ex2")
        for ps, rhs in ((ps_mu, x1bf), (ps_ex2, sq)):
            for st, ln in chunks512(0, T):
                for kt in range(2):
                    nc.tensor.matmul(ps[:, st:st + ln], ones_stat,
                                     rhs[kt][:, st:st + ln],
                                     start=(kt == 0), stop=(kt == 1))
        t2m = work.tile([128, T], F32, tag="l2_t2m", bufs=1, name=f"{tag}_t2m")
        t2 = work.tile([128, T], F32, tag="l2_t2", bufs=2, name=f"{tag}_t2")
        lnv = work.tile([128, T], F32, tag="l2_lnv", bufs=1, name=f"{tag}_lnv")
        alpha = work.tile([128, T], F32, tag="l2_al", bufs=2, name=f"{tag}_al")
        slot()
        nc.scalar.activation(out=t2m, in_=ps_mu, func=AF.Square)
        nc.vector.tensor_tensor(out=t2, in0=ps_ex2, in1=t2m, op=ALU.subtract)
        nc.scalar.activation(out=lnv, in_=t2, func=AF.Ln, bias=eps_sb,
                             scale=1.0)
        nc.scalar.activation(out=alpha, in_=lnv, func=AF.Exp, scale=-0.5)
        slot()
        h2 = []
        for ct in range(2):
            hf = work.tile([128, T], F32, tag=f"l2_hf{ct}", bufs=1,
                           name=f"{tag}_hf{ct}")
            hh_sb = work.tile([128, T], BF16, tag=f"l2_h{ct}", bufs=2,
                              name=f"{tag}_h{ct}")
            nc.vector.tensor_tensor(out=hf, in0=x1[ct], in1=ps_mu,
                                    op=ALU.subtract)
            nc.vector.tensor_tensor(out=hh_sb, in0=hf, in1=alpha,
                                    op=ALU.mult)
            h2.append(hh_sb)

        # MLP: ps_u through 1-bank pss chunks, ps_y accumulators in pa
        ps_y = [pa_tile(f"ps_y{mt}_{b}") for mt in range(2)]
        for f in range(8):
            ut = work.tile([128, T], BF16, tag="ut", bufs=3,
                           name=f"ut{f}_{b}")
            for ci, (st, ln) in enumerate(chunks512(0, T)):
                ps_u = pss_tile(ln, f"ps_u{f}_{ci}_{b}")
                for kt in range(2):
                    nc.tensor.matmul(
                        ps_u[:, 0:ln],
                        w1_sb[kt][:, f * 128:(f + 1) * 128],
                        h2[kt][:, st:st + ln],
                        start=(kt == 0), stop=(kt == 1))
                nc.vector.tensor_scalar(out=ut[:, st:st + ln],
                                        in0=ps_u[:, 0:ln],
                                        scalar1=b1_sb[:, f:f + 1],
                                        scalar2=0.0,
                                        op0=ALU.add, op1=ALU.max)
            for mt in range(2):
                for st, ln in chunks512(0, T):
                    nc.tensor.matmul(
                        ps_y[mt][:, st:st + ln],
                        w2_sb[f][:, mt * 128:(mt + 1) * 128],
                        ut[:, st:st + ln],
                        start=(f == 0), stop=False)
            slot()
        while oi[0] < len(overlap):
            overlap[oi[0]]()
            oi[0] += 1
        yt = [work.tile([128, T], F32, tag=f"yt{ct}", bufs=2,
                        name=f"yt{ct}_{b}") for ct in range(2)]
        for mt in range(2):
            for st, ln in chunks512(0, T):
                nc.tensor.matmul(
                    ps_y[mt][:, st:st + ln],
                    b2_sb[0:1, mt * 128:(mt + 1) * 128],
                    ones_row[:, st:st + ln],
                    start=False, stop=True)
            nc.vector.tensor_tensor(out=yt[mt], in0=ps_y[mt], in1=x1[mt],
                                    op=ALU.add)
            nc.sync.dma_start(out=yt_d[b, mt * 128:(mt + 1) * 128, :],
                              in_=yt[mt])

    # ---- software pipeline over the 4 batches ----
    for piece in front_pieces(0):
        piece()
    for b in range(B_PER_CORE):
        with nc.named_scope(f"attn_{b}"):
            emit_attention(b)
        with nc.named_scope(f"back_{b}"):
            emit_back(b, front_pieces(b + 1))


_NC_CACHE = None


def _prep_weights(Wq, Wk, Wv, Wo, bo, W1, b1, W2, b2, g1, be1, g2, be2):
    f64 = np.float64
    g1, be1 = g1.astype(f64), be1.astype(f64)
    g2, be2 = g2.astype(f64), be2.astype(f64)

    def fold_qkv(W):  # W: [H, C, HS] -> folded [C, H*HS], bias [H*HS]
        Wf = W.astype(f64) * g1[None, :, None]
        Wcat = np.concatenate([Wf[h] for h in range(H)], axis=1)  # [C, 256]
        bias = np.concatenate([be1 @ Wf[h] for h in range(H)])  # [256]
        return Wcat, bias

    WqF, bq = fold_qkv(Wq)
    WkF, bk = fold_qkv(Wk)
    WvF, bv = fold_qkv(Wv)
    # h2 = z*g2 + be2 ; relu(h2@W1 + b1) = relu(z @ (g2*W1) + (be2@W1 + b1))
    W1F = W1.astype(f64) * g2[:, None]
    b1F = b1.astype(f64) + be2 @ W1.astype(f64)

    def f32(a):
        return np.ascontiguousarray(a, dtype=np.float32)

    def bf16(a):
        return np.ascontiguousarray(np.asarray(a, f64).astype(ml_dtypes.bfloat16))

    return {
        "wq": bf16(WqF.reshape(2, 128, C)),
        "wk": bf16(WkF.reshape(2, 128, C)),
        "wv": bf16(WvF.reshape(2, 128, C)),
        "wo": bf16(np.asarray(Wo, f64).reshape(2, 128, C)),
        "w1": bf16(W1F.reshape(2, 128, F)),
        "w2": bf16(np.asarray(W2, f64).reshape(8, 128, C)),
        "bq": f32(bq.reshape(2, 128).T),
        "bk": f32(bk.reshape(2, 128).T),
        "bv": f32(np.broadcast_to(bv, (128, C))),
        "bo": bf16(np.asarray(bo, f64).reshape(1, C)),
        "b1": f32(b1F.reshape(8, 128).T),
        "b2": bf16(np.asarray(b2, f64).reshape(1, C)),
        "mask": bf16(np.triu(np.ones((128, 128)))),
        "ones_c": bf16(np.full((128, 128), 1.0 / C)),
        "ones_t": bf16(np.ones((1, T))),
        "ones_va": bf16(np.ones((128, H, 64))),
    }


def kernel(x, Wq, Wk, Wv, Wo, bo, W1, b1, W2, b2, g1, be1, g2, be2,
           _trace=False):
    global _NC_CACHE
    if _NC_CACHE is None:
        _NC_CACHE = _build_nc()
    nc = _NC_CACHE

    x = np.asarray(x, dtype=np.float32)
    weights = _prep_weights(
        np.asarray(Wq), np.asarray(Wk), np.asarray(Wv), np.asarray(Wo),
        np.asarray(bo), np.asarray(W1), np.asarray(b1), np.asarray(W2),
        np.asarray(b2), np.asarray(g1), np.asarray(be1), np.asarray(g2),
        np.asarray(be2))
    xt = np.ascontiguousarray(x.transpose(0, 2, 1))  # [B, C, T]
    xbf = xt.astype(ml_dtypes.bfloat16)

    in_maps = []
    for core in range(N_CORES):
        m = dict(weights)
        sl = slice(core * B_PER_CORE, (core + 1) * B_PER_CORE)
        m["xt"] = np.ascontiguousarray(xt[sl])
        m["xbf"] = np.ascontiguousarray(xbf[sl])
        in_maps.append(m)

    res = run_bass_kernel_spmd(nc, in_maps, list(range(N_CORES)),
                               trace=_trace)
    outs = [res.results[i]["yt"] for i in range(N_CORES)]  # [4, C, T] each
    y = np.concatenate(outs, axis=0).transpose(0, 2, 1)  # [B, T, C]
    if _trace:
        kernel.last_exec_time_ns = res.exec_time_ns
        kernel.last_results = res
    return np.ascontiguousarray(y)
